# revision 32
# baseline (speedup 1.0000x reference)
"""nn_Decoder (LSTM decoder) Trainium2 Bass kernel, 8-core tensor-parallel,
two phase-shifted batch streams.

Strategy (hardcoded for B=64, L=128, H=1024, O=1, T=256, 8 cores):
  The 4H=4096 gate rows are sharded 8 ways: each core owns a 128-row H-slice
  of each gate (layout f|i|o|g), computes gates transposed on PE (W_hh^T
  blocks stationary in bf16, h^T streamed), does the cell elementwise on
  ACT/DVE, and broadcasts its h^T chunk to all peers each step via one
  8-destination remote_dma_broadcast.

  The batch is split into two independent 32-wide streams (A = batch 0:32,
  B = 32:64) running the same recurrence phase-shifted by ~half a step.
  While stream A's h-broadcast is in flight, the engines process stream B's
  matmuls / cell, and vice versa — the exchange latency (~2us: trigger 0.33,
  SWDGE doorbell->drain 0.67, per-engine descriptor drain 0.6-0.8, link) is
  partly hidden behind real work, and the high PE duty keeps the PE p-state
  at 2.4 GHz (it throttles after any idle). Both streams' broadcast frames
  share SWDGE queue 0 in strict A,B alternation (a second SWDGE queue
  silently corrupts transfers on this runtime).

  Measured on 8 axon trn2 cores: ~1.63 ms total (~6.4 us/step, from 1.86 ms
  baseline), rel err 2.1e-3. Traced clean-phase rounds run at ~4.8-5.1 us;
  the mean includes residual cross-core phase oscillation.

  Other latency tricks carried over:
  - Per-SOURCE arrival semaphores (sender passes remote_sem=s_src[my_id]
    inside its Switch case): each receiver's PE consumes h chunks as they
    arrive instead of waiting for all 8.
  - The g-gate tanh is folded into the gate sigmoid: host prep scales the
    g rows of W_ih/W_hh/bias by 2, the device does ONE sigmoid over all
    gate columns (tanh(x) = 2*sigmoid(2x)-1), and DVE fixes g up with a
    single fused tensor_scalar (*2 - 1).
  - Cell state + temporaries in SBUF (DVE SBUF access 58 cyc vs 120 PSUM).
  - x_gates precomputed once, re-injected into each step's PSUM accumulation
    via identity matmuls (bf16 hi+lo split, error ~2^-18).
"""

import numpy as np
import ml_dtypes

B, L, H, O, T = 64, 128, 1024, 1, 256
NC = 8
NPH = 4
# device gate-column order f|g|i|o (indices into pytorch's i,f,g,o row blocks):
# f and g first so the sigmoid splits into sig(f,g) -> DVE t1/gt starts early
# while sig(i,o) still runs on ACT.
GATE_ORDER = [1, 2, 0, 3]
# per-stream gate slices in the [128, 128] gates tile (4 gates x 32 batch)
SG_F = slice(0, 32)
SG_G = slice(32, 64)
SG_I = slice(64, 96)
SG_O = slice(96, 128)
# Filler matmuls per stream section. Besides keeping the PE p-state warm,
# they are sized so the PE is the (slightly) binding resource of each round:
# PE-paced rounds run in lockstep across all 8 cores, which kills the
# chain-bound phase oscillation (cores drifting apart and waiting on the
# slowest peer's h-broadcast costs ~1.3us/step on average).
FILLER_N = 128
FILL_A = 3
FILL_B = 3

_cache = {}


def _build_lstm_nc(T_steps=T, solo=False, detect_races=True):
    import concourse.bacc as bacc
    import concourse.bass as bass
    import concourse.mybir as mybir

    dt = mybir.dt
    AF = mybir.ActivationFunctionType
    ALU = mybir.AluOpType
    Tn = T_steps

    nc = bacc.Bacc(
        None,
        target_bir_lowering=False,
        debug=False,
        num_devices=NC,
        detect_race_conditions=detect_races,
    )

    d_latT = nc.dram_tensor("latT", [128, 64], dt.float32, kind="ExternalInput")
    d_WlinT = nc.dram_tensor("WlinT", [128, 1024], dt.float32, kind="ExternalInput")
    d_blinT = nc.dram_tensor("blinT", [128, 8], dt.float32, kind="ExternalInput")
    d_WihT = nc.dram_tensor("WihT", [128, 4096], dt.float32, kind="ExternalInput")
    d_bg = nc.dram_tensor("bg", [1, 512], dt.float32, kind="ExternalInput")
    d_ones = nc.dram_tensor("ones1", [1, 64], dt.float32, kind="ExternalInput")
    d_WhT = nc.dram_tensor("WhT", [128, 4096], dt.bfloat16, kind="ExternalInput")
    d_wout = nc.dram_tensor("wout", [128, 1], dt.bfloat16, kind="ExternalInput")
    d_I64 = nc.dram_tensor("I64", [64, 64], dt.bfloat16, kind="ExternalInput")
    d_out = nc.dram_tensor("outp", [64, Tn], dt.float32, kind="ExternalOutput")
    N_IN = 9

    s_latT = nc.alloc_sbuf_tensor("s_latT", [128, 64], dt.float32)
    s_WlinT = nc.alloc_sbuf_tensor("s_WlinT", [128, 1024], dt.float32)
    s_blinT = nc.alloc_sbuf_tensor("s_blinT", [128, 8], dt.float32)
    s_WihT = nc.alloc_sbuf_tensor("s_WihT", [128, 4096], dt.float32)
    s_bg = nc.alloc_sbuf_tensor("s_bg", [1, 512], dt.float32)
    s_ones = nc.alloc_sbuf_tensor("s_ones", [1, 64], dt.float32)
    s_WhT = nc.alloc_sbuf_tensor("s_WhT", [128, 4096], dt.bfloat16)
    s_wout = nc.alloc_sbuf_tensor("s_wout", [128, 1], dt.bfloat16)
    s_I64 = nc.alloc_sbuf_tensor("s_I64", [64, 64], dt.bfloat16)

    s_hidT = nc.alloc_sbuf_tensor("s_hidT", [128, 512], dt.float32)
    s_Xhi = nc.alloc_sbuf_tensor("s_Xhi", [64, 512], dt.bfloat16)
    s_Xlo = nc.alloc_sbuf_tensor("s_Xlo", [64, 512], dt.bfloat16)
    s_Xres = nc.alloc_sbuf_tensor("s_Xres", [64, 512], dt.float32)
    # per-stream recv buffers: 4-deep rotation, 8 slots x 32 batch cols
    recvA = [
        nc.alloc_sbuf_tensor(f"recvA{p}", [128, 256], dt.bfloat16) for p in range(NPH)
    ]
    recvB = [
        nc.alloc_sbuf_tensor(f"recvB{p}", [128, 256], dt.bfloat16) for p in range(NPH)
    ]
    gA = [nc.alloc_sbuf_tensor(f"gA{p}", [128, 128], dt.float32) for p in range(2)]
    gB = [nc.alloc_sbuf_tensor(f"gB{p}", [128, 128], dt.float32) for p in range(2)]
    thA = [nc.alloc_sbuf_tensor(f"thA{p}", [128, 32], dt.float32) for p in range(2)]
    thB = [nc.alloc_sbuf_tensor(f"thB{p}", [128, 32], dt.float32) for p in range(2)]
    # each send buffer padded to its own 512B-aligned footprint
    _hsA = [
        nc.alloc_sbuf_tensor(f"h_sendA{p}", [128, 256], dt.bfloat16) for p in range(2)
    ]
    _hsB = [
        nc.alloc_sbuf_tensor(f"h_sendB{p}", [128, 256], dt.bfloat16) for p in range(2)
    ]
    h_sendA = [t[:, 0:32] for t in _hsA]
    h_sendB = [t[:, 0:32] for t in _hsB]
    s_t1 = nc.alloc_sbuf_tensor("s_t1", [128, 32], dt.float32)
    s_t2 = nc.alloc_sbuf_tensor("s_t2", [128, 32], dt.float32)
    s_gt = nc.alloc_sbuf_tensor("s_gt", [128, 32], dt.float32)
    cA = [nc.alloc_sbuf_tensor(f"cA{p}", [128, 32], dt.float32) for p in range(2)]
    cB = [nc.alloc_sbuf_tensor(f"cB{p}", [128, 32], dt.float32) for p in range(2)]
    s_out = nc.alloc_sbuf_tensor("s_out", [64, Tn], dt.float32)

    p_hid = nc.alloc_psum_tensor("p_hid", [128, 512], dt.float32)
    p_x = nc.alloc_psum_tensor("p_x", [128, 512], dt.float32)
    p_gA = [
        nc.alloc_psum_tensor(f"p_gA{p}", [128, 512], dt.float32) for p in range(2)
    ]
    p_gB = [
        nc.alloc_psum_tensor(f"p_gB{p}", [128, 512], dt.float32) for p in range(2)
    ]
    p_out = nc.alloc_psum_tensor("p_out", [128, 512], dt.float32)
    p_fill = nc.alloc_psum_tensor("p_fill", [128, 512], dt.float32)

    s_srcA = [nc.alloc_semaphore(f"s_srcA{j}") for j in range(NC)]
    s_srcB = [nc.alloc_semaphore(f"s_srcB{j}") for j in range(NC)]
    s_peA = nc.alloc_semaphore("s_peA")
    s_peB = nc.alloc_semaphore("s_peB")
    s_sigA = nc.alloc_semaphore("s_sigA")
    s_sigB = nc.alloc_semaphore("s_sigB")
    s_thsA = nc.alloc_semaphore("s_thsA")
    s_thsB = nc.alloc_semaphore("s_thsB")
    s_cA = nc.alloc_semaphore("s_cA")
    s_cB = nc.alloc_semaphore("s_cB")
    s_hA = nc.alloc_semaphore("s_hA")
    s_hB = nc.alloc_semaphore("s_hB")
    s_locA = nc.alloc_semaphore("s_locA")
    s_locB = nc.alloc_semaphore("s_locB")
    s_prepA = nc.alloc_semaphore("s_prepA")
    s_prepB = nc.alloc_semaphore("s_prepB")
    s_ph = nc.alloc_semaphore("s_ph")
    s_v = nc.alloc_semaphore("s_v")
    s_xrdy = nc.alloc_semaphore("s_xrdy")
    s_osem = nc.alloc_semaphore("s_osem")
    s_fin = nc.alloc_semaphore("s_fin")
    dma_sem = nc.alloc_semaphore("dma_sem")

    SRC_INC = 16 if solo else 2

    # X-inject: out pg[:, 32m:32m+32] = Xhi[batch j, 128m+i] for this stream.
    def x_inject(tensor, pg, brow, istart, final_stop=False):
        for m in range(4):
            tensor.matmul(
                pg[:, 32 * m : 32 * m + 32],
                s_Xhi[brow, 128 * m : 128 * m + 128],
                s_I64[brow, istart],
                start=(m == 0),
                stop=False,
            )
            mm = tensor.matmul(
                pg[:, 32 * m : 32 * m + 32],
                s_Xlo[brow, 128 * m : 128 * m + 128],
                s_I64[brow, istart],
                start=False,
                stop=(final_stop and m == 3),
            )
        return mm

    def fillers(tensor, n):
        for fi in range(n):
            tensor.matmul(
                p_fill[:, 0:FILLER_N],
                s_WhT[:, 0:128],
                s_WhT[:, 128 : 128 + FILLER_N],
                start=(fi == 0),
                stop=(fi == n - 1),
            )

    BROW_A, IST_A = slice(0, 32), slice(0, 32)
    BROW_B, IST_B = slice(32, 64), slice(32, 64)

    with nc.Block() as block:

        @block.sync
        def _(sync: bass.BassEngine):
            for d, s in [
                (d_latT, s_latT),
                (d_WlinT, s_WlinT),
                (d_blinT, s_blinT),
                (d_WihT, s_WihT),
                (d_bg, s_bg),
                (d_ones, s_ones),
                (d_WhT, s_WhT),
                (d_wout, s_wout),
                (d_I64, s_I64),
            ]:
                sync.dma_start(s[:, :], d[:, :]).then_inc(dma_sem, 16)
            sync.wait_ge(s_fin, 1)
            sync.dma_start(d_out[:, :], s_out[:, :]).then_inc(dma_sem, 16)
            sync.wait_ge(dma_sem, 16 * (N_IN + 1))

        @block.tensor
        def _(tensor: bass.BassTensorEngine):
            tensor.wait_ge(dma_sem, 16 * N_IN)
            # phase 1a: hidden^T chunks = W_lin row-chunks @ latent^T
            for m in range(8):
                mm = tensor.matmul(
                    p_hid[:, 64 * m : 64 * m + 64],
                    s_WlinT[:, 128 * m : 128 * m + 128],
                    s_latT[:, :],
                    start=True,
                    stop=True,
                )
            mm.then_inc(s_ph, 1)  # s_ph = 1
            # phase 1b: x_gates (B-major) = hidden @ W_ih_slice^T + bias
            tensor.wait_ge(s_ph, 2)
            for k in range(8):
                tensor.matmul(
                    p_x[0:64, :],
                    s_hidT[:, 64 * k : 64 * k + 64],
                    s_WihT[:, 512 * k : 512 * k + 512],
                    start=(k == 0),
                    stop=False,
                )
            mm = tensor.matmul(
                p_x[0:64, :], s_ones[0:1, :], s_bg[0:1, :], start=False, stop=True
            )
            mm.then_inc(s_ph, 1)  # s_ph = 3
            # HAM warmup
            for fi in range(12):
                tensor.matmul(
                    p_fill[:, 0:512],
                    s_WhT[:, 0:128],
                    s_WhT[:, 128:640],
                    start=(fi == 0),
                    stop=(fi == 11),
                )
            # prologue: round-0 gates = X only
            tensor.wait_ge(s_xrdy, 1)
            x_inject(tensor, p_gA[0], BROW_A, IST_A, final_stop=True).then_inc(
                s_peA, 1
            )
            x_inject(tensor, p_gB[0], BROW_B, IST_B, final_stop=True).then_inc(
                s_peB, 1
            )

            for r in range(Tn):
                # Both streams' recv blocks come FIRST: recvB's stop gates
                # sigma_B, and B's chunks have usually already arrived, so
                # running recvB right after recvA (instead of behind
                # XA/woutA/fillers) fires B's cell + broadcast ~0.8us
                # earlier, growing stream B's slack so it never paces the
                # round.
                if r >= 1:
                    par = r % NPH
                    pg = p_gA[r % 2]
                    for x in range(8):
                        tensor.wait_ge(s_srcA[x], SRC_INC * r)
                        for m in range(4):
                            mm = tensor.matmul(
                                pg[:, 32 * m : 32 * m + 32],
                                s_WhT[:, (4 * x + m) * 128 : (4 * x + m + 1) * 128],
                                recvA[par][:, 32 * x : 32 * x + 32],
                                start=False,
                                stop=(x == 7 and m == 3),
                            )
                    mm.then_inc(s_peA, 1)  # r+1
                    pg = p_gB[r % 2]
                    for x in range(8):
                        tensor.wait_ge(s_srcB[x], SRC_INC * r)
                        for m in range(4):
                            mm = tensor.matmul(
                                pg[:, 32 * m : 32 * m + 32],
                                s_WhT[:, (4 * x + m) * 128 : (4 * x + m + 1) * 128],
                                recvB[par][:, 32 * x : 32 * x + 32],
                                start=False,
                                stop=(x == 7 and m == 3),
                            )
                    mm.then_inc(s_peB, 1)  # r+1
                if r + 1 < Tn:
                    # X for round r+1 opens the pg[(r+1)%2] accumulation group
                    x_inject(tensor, p_gA[(r + 1) % 2], BROW_A, IST_A)
                if r >= 1:
                    tensor.wait_ge(s_hA, r)
                    tensor.matmul(
                        p_out[0:32, r - 1 : r],
                        h_sendA[r % 2],
                        s_wout[:, 0:1],
                        start=True,
                        stop=True,
                    )
                fillers(tensor, FILL_A)
                if r + 1 < Tn:
                    x_inject(tensor, p_gB[(r + 1) % 2], BROW_B, IST_B)
                if r >= 1:
                    tensor.wait_ge(s_hB, r)
                    tensor.matmul(
                        p_out[32:64, r - 1 : r],
                        h_sendB[r % 2],
                        s_wout[:, 0:1],
                        start=True,
                        stop=True,
                    )
                fillers(tensor, FILL_B)

            tensor.wait_ge(s_hA, Tn)
            tensor.matmul(
                p_out[0:32, Tn - 1 : Tn],
                h_sendA[Tn % 2],
                s_wout[:, 0:1],
                start=True,
                stop=True,
            ).then_inc(s_osem, 1)
            tensor.wait_ge(s_hB, Tn)
            tensor.matmul(
                p_out[32:64, Tn - 1 : Tn],
                h_sendB[Tn % 2],
                s_wout[:, 0:1],
                start=True,
                stop=True,
            ).then_inc(s_osem, 1)

        @block.scalar
        def _(scalar: bass.BassScalarEngine):
            scalar.wait_ge(s_ph, 1)
            for m in range(8):
                a = scalar.activation(
                    s_hidT[:, 64 * m : 64 * m + 64],
                    p_hid[:, 64 * m : 64 * m + 64],
                    AF.Identity,
                    bias=s_blinT[:, m : m + 1],
                    scale=1.0,
                )
            a.then_inc(s_ph, 1)  # s_ph = 2
            for r in range(Tn):
                scalar.wait_ge(s_peA, r + 1)
                # sig(f,g) first so DVE can start t1/gt while sig(i,o) runs
                scalar.activation(
                    gA[r % 2][:, 0:64], p_gA[r % 2][:, 0:64], AF.Sigmoid
                ).then_inc(s_sigA, 1)  # 2r+1
                scalar.activation(
                    gA[r % 2][:, 64:128], p_gA[r % 2][:, 64:128], AF.Sigmoid
                ).then_inc(s_sigA, 1)  # 2r+2
                scalar.wait_ge(s_cA, r + 1)
                scalar.activation(
                    thA[r % 2][:, :], cA[r % 2][:, :], AF.Tanh
                ).then_inc(s_thsA, 1)  # r+1
                scalar.wait_ge(s_peB, r + 1)
                scalar.activation(
                    gB[r % 2][:, 0:64], p_gB[r % 2][:, 0:64], AF.Sigmoid
                ).then_inc(s_sigB, 1)  # 2r+1
                scalar.activation(
                    gB[r % 2][:, 64:128], p_gB[r % 2][:, 64:128], AF.Sigmoid
                ).then_inc(s_sigB, 1)  # 2r+2
                scalar.wait_ge(s_cB, r + 1)
                scalar.activation(
                    thB[r % 2][:, :], cB[r % 2][:, :], AF.Tanh
                ).then_inc(s_thsB, 1)  # r+1
            scalar.wait_ge(s_osem, 2)
            scalar.activation(s_out[:, :], p_out[0:64, 0:Tn], AF.Copy).then_inc(
                s_fin, 1
            )

        @block.vector
        def _(vector: bass.BassVectorEngine):
            vector.wait_ge(s_ph, 3)
            vector.tensor_copy(s_Xhi[:, :], p_x[0:64, :]).then_inc(s_v, 1)
            vector.wait_ge(s_v, 1)
            vector.tensor_tensor(
                s_Xres[0:64, :], p_x[0:64, :], s_Xhi[:, :], ALU.subtract
            ).then_inc(s_v, 1)
            vector.wait_ge(s_v, 2)
            vector.tensor_copy(s_Xlo[:, :], s_Xres[0:64, :])
            vector.memset(cA[1][:, :], 0.0)
            vector.memset(cB[1][:, :], 0.0).then_inc(s_xrdy, 1)
            # intra-DVE RAW edges (gt->t2, t2->c) carry explicit self-sems:
            # back-to-back DVE ops can read an operand before the prior op's
            # write fully lands (seen as stream-A corruption without these;
            # relying on op-order spacing alone also fails on HW).
            for r in range(Tn):
                # ---- stream A cell ----
                g = gA[r % 2]
                vector.wait_ge(s_sigA, 2 * r + 1)  # sig(f,g)
                vector.tensor_tensor(
                    s_t1[:, :], g[:, SG_F], cA[(r + 1) % 2][:, :], ALU.mult
                ).then_inc(s_v, 1)  # 6r+3
                vector.tensor_scalar(
                    s_gt[:, :], g[:, SG_G], 2.0, -1.0, ALU.mult, ALU.add
                ).then_inc(s_v, 1)  # 6r+4
                vector.wait_ge(s_sigA, 2 * r + 2)  # sig(i,o)
                vector.wait_ge(s_v, 6 * r + 4)
                vector.tensor_tensor(
                    s_t2[:, :], g[:, SG_I], s_gt[:, :], ALU.mult
                ).then_inc(s_v, 1)  # 6r+5
                vector.wait_ge(s_v, 6 * r + 5)
                vector.tensor_tensor(
                    cA[r % 2][:, :], s_t1[:, :], s_t2[:, :], ALU.add
                ).then_inc(s_cA, 1)  # r+1
                vector.wait_ge(s_thsA, r + 1)
                if r >= 2 and not solo:
                    vector.wait_ge(s_locA, 16 * (r - 1))
                vector.tensor_tensor(
                    h_sendA[(r + 1) % 2], g[:, SG_O], thA[r % 2][:, :], ALU.mult
                ).then_inc(s_hA, 1)  # r+1
                # ---- stream B cell ----
                g = gB[r % 2]
                vector.wait_ge(s_sigB, 2 * r + 1)
                vector.tensor_tensor(
                    s_t1[:, :], g[:, SG_F], cB[(r + 1) % 2][:, :], ALU.mult
                ).then_inc(s_v, 1)  # 6r+6
                vector.tensor_scalar(
                    s_gt[:, :], g[:, SG_G], 2.0, -1.0, ALU.mult, ALU.add
                ).then_inc(s_v, 1)  # 6r+7
                vector.wait_ge(s_sigB, 2 * r + 2)
                vector.wait_ge(s_v, 6 * r + 7)
                vector.tensor_tensor(
                    s_t2[:, :], g[:, SG_I], s_gt[:, :], ALU.mult
                ).then_inc(s_v, 1)  # 6r+8
                vector.wait_ge(s_v, 6 * r + 8)
                vector.tensor_tensor(
                    cB[r % 2][:, :], s_t1[:, :], s_t2[:, :], ALU.add
                ).then_inc(s_cB, 1)  # r+1
                vector.wait_ge(s_thsB, r + 1)
                if r >= 2 and not solo:
                    vector.wait_ge(s_locB, 16 * (r - 1))
                vector.tensor_tensor(
                    h_sendB[(r + 1) % 2], g[:, SG_O], thB[r % 2][:, :], ALU.mult
                ).then_inc(s_hB, 1)  # r+1

        @block.gpsimd
        def _(gpsimd: bass.BassGpSimd):
            if solo:
                for r in range(Tn):
                    gpsimd.wait_ge(s_hA, r + 1)
                    for j in range(8):
                        gpsimd.dma_start(
                            recvA[(r + 1) % NPH][:, 32 * j : 32 * j + 32],
                            h_sendA[(r + 1) % 2],
                        ).then_inc(s_srcA[j], 16)
                    gpsimd.wait_ge(s_hB, r + 1)
                    for j in range(8):
                        gpsimd.dma_start(
                            recvB[(r + 1) % NPH][:, 32 * j : 32 * j + 32],
                            h_sendB[(r + 1) % 2],
                        ).then_inc(s_srcB[j], 16)
                return
            gpsimd.bir_kernel_barrier_wait([list(range(NC))])
            pid = gpsimd.partition_id()
            for case in gpsimd.Switch(pid, NC):
                rdests = [(0, j) for j in range(NC)]
                # prologue: prep round-0 frames (A then B, strict FIFO order)
                gpsimd.remote_dma_broadcast(
                    out_ap=recvA[1][:, 32 * case : 32 * case + 32],
                    in_ap=h_sendA[1],
                    remote_sem=s_srcA[case],
                    local_sem=s_locA,
                    rdests=rdests,
                ).then_inc(s_prepA, 1)
                gpsimd.remote_dma_broadcast(
                    out_ap=recvB[1][:, 32 * case : 32 * case + 32],
                    in_ap=h_sendB[1],
                    remote_sem=s_srcB[case],
                    local_sem=s_locB,
                    rdests=rdests,
                ).then_inc(s_prepB, 1)
                for r in range(Tn):
                    gpsimd.wait_ge(s_prepA, r + 1)
                    gpsimd.wait_ge(s_hA, r + 1)
                    gpsimd.trigger_dma(count=1)  # fires frame A(r)
                    gpsimd.wait_ge(s_prepB, r + 1)
                    gpsimd.wait_ge(s_hB, r + 1)
                    gpsimd.trigger_dma(count=1)  # fires frame B(r)
                    if r + 1 < Tn:
                        gpsimd.remote_dma_broadcast(
                            out_ap=recvA[(r + 2) % NPH][
                                :, 32 * case : 32 * case + 32
                            ],
                            in_ap=h_sendA[(r + 2) % 2],
                            remote_sem=s_srcA[case],
                            local_sem=s_locA,
                            rdests=rdests,
                        ).then_inc(s_prepA, 1)
                        gpsimd.remote_dma_broadcast(
                            out_ap=recvB[(r + 2) % NPH][
                                :, 32 * case : 32 * case + 32
                            ],
                            in_ap=h_sendB[(r + 2) % 2],
                            remote_sem=s_srcB[case],
                            local_sem=s_locB,
                            rdests=rdests,
                        ).then_inc(s_prepB, 1)
                    gpsimd.wait_ge(s_locA, 16 * r)
                    gpsimd.wait_ge(s_locB, 16 * r)

    nc.has_collectives = not solo
    nc.finalize()
    return nc


def _prep_core_inputs(inputs: dict, r: int, src_row=None) -> dict:
    if src_row is None:
        src_row = list(range(8))  # slot j holds logical core j's H-chunk
    f32 = np.float32
    bf16 = ml_dtypes.bfloat16
    latent = np.asarray(inputs["latent"], f32)
    W_lin = np.asarray(inputs["W_lin"], f32)
    b_lin = np.asarray(inputs["b_lin"], f32)
    W_ih = np.asarray(inputs["W_ih"], f32)
    W_hh = np.asarray(inputs["W_hh"], f32)
    b_ih = np.asarray(inputs["b_ih"], f32)
    b_hh = np.asarray(inputs["b_hh"], f32)
    W_out = np.asarray(inputs["W_out"], f32)

    HS = 128
    sl = slice(HS * r, HS * (r + 1))

    # g-gate (pytorch index 2) rows scaled by 2: tanh(x) = 2*sigmoid(2x)-1,
    # so the device applies one sigmoid to all four gates and DVE fixes g up.
    gscale = {2: 2.0}

    Wih_sl = np.concatenate(
        [
            gscale.get(g, 1.0) * W_ih[g * H + HS * r : g * H + HS * (r + 1), :]
            for g in GATE_ORDER
        ],
        axis=0,
    )
    WihT = Wih_sl.T.reshape(8, 128, 512).transpose(1, 0, 2).reshape(128, 4096).copy()

    bgv = b_ih + b_hh
    bg = np.concatenate(
        [
            gscale.get(g, 1.0) * bgv[g * H + HS * r : g * H + HS * (r + 1)]
            for g in GATE_ORDER
        ]
    ).reshape(1, 512)

    WhT = np.zeros((128, 4096), f32)
    for x in range(8):
        srcc = src_row[x]
        for m, g in enumerate(GATE_ORDER):
            blk = W_hh[
                g * H + HS * r : g * H + HS * (r + 1), HS * srcc : HS * (srcc + 1)
            ]
            WhT[:, (4 * x + m) * 128 : (4 * x + m + 1) * 128] = gscale.get(
                g, 1.0
            ) * blk.T

    return {
        "latT": np.ascontiguousarray(latent.T),
        "WlinT": np.ascontiguousarray(W_lin.T),
        "blinT": np.ascontiguousarray(b_lin.reshape(8, 128).T),
        "WihT": WihT,
        "bg": bg,
        "ones1": np.ones((1, 64), f32),
        "WhT": WhT.astype(bf16),
        "wout": np.ascontiguousarray(W_out[0, sl].reshape(128, 1)).astype(bf16),
        "I64": np.eye(64, dtype=f32).astype(bf16),
    }


def _run(inputs: dict, trace: bool = False):
    from concourse.bass_utils import run_bass_kernel_spmd

    if "nc" not in _cache:
        _cache["nc"] = _build_lstm_nc(T)
    nc = _cache["nc"]
    in_maps = [_prep_core_inputs(inputs, r) for r in range(NC)]
    res = run_bass_kernel_spmd(
        nc, in_maps, core_ids=list(range(NC)), trace=trace
    )
    outs = [np.asarray(res.results[r]["outp"], np.float64) for r in range(NC)]
    b_out = np.asarray(inputs["b_out"], np.float64)
    total = outs[0]
    for o in outs[1:]:
        total = total + o
    total = total + b_out[0]
    out = total[:, :, None].astype(np.float32)
    return out, res


def kernel(**inputs) -> np.ndarray:
    seq_len = int(inputs.get("seq_len", T))
    assert seq_len == T, f"kernel hardcoded for seq_len={T}, got {seq_len}"
    out, _ = _run(inputs, trace=False)
    return out


# revision 34
# speedup vs baseline: 1.0264x; 1.0264x over previous
"""nn_Decoder (LSTM decoder) Trainium2 Bass kernel, 8-core tensor-parallel,
two phase-shifted batch streams.

Strategy (hardcoded for B=64, L=128, H=1024, O=1, T=256, 8 cores):
  The 4H=4096 gate rows are sharded 8 ways: each core owns a 128-row H-slice
  of each gate (layout f|i|o|g), computes gates transposed on PE (W_hh^T
  blocks stationary in bf16, h^T streamed), does the cell elementwise on
  ACT/DVE, and broadcasts its h^T chunk to all peers each step via one
  8-destination remote_dma_broadcast.

  The batch is split into two independent 32-wide streams (A = batch 0:32,
  B = 32:64) running the same recurrence phase-shifted by ~half a step.
  While stream A's h-broadcast is in flight, the engines process stream B's
  matmuls / cell, and vice versa — the exchange latency (~2us: trigger 0.33,
  SWDGE doorbell->drain 0.67, per-engine descriptor drain 0.6-0.8, link) is
  partly hidden behind real work, and the high PE duty keeps the PE p-state
  at 2.4 GHz (it throttles after any idle). Both streams' broadcast frames
  share SWDGE queue 0 in strict A,B alternation (a second SWDGE queue
  silently corrupts transfers on this runtime).

  Measured on 8 axon trn2 cores: ~1.63 ms total (~6.4 us/step, from 1.86 ms
  baseline), rel err 2.1e-3. Traced clean-phase rounds run at ~4.8-5.1 us;
  the mean includes residual cross-core phase oscillation.

  Other latency tricks carried over:
  - Per-SOURCE arrival semaphores (sender passes remote_sem=s_src[my_id]
    inside its Switch case): each receiver's PE consumes h chunks as they
    arrive instead of waiting for all 8.
  - The g-gate tanh is folded into the gate sigmoid: host prep scales the
    g rows of W_ih/W_hh/bias by 2, the device does ONE sigmoid over all
    gate columns (tanh(x) = 2*sigmoid(2x)-1), and DVE fixes g up with a
    single fused tensor_scalar (*2 - 1).
  - Cell state + temporaries in SBUF (DVE SBUF access 58 cyc vs 120 PSUM).
  - x_gates precomputed once, re-injected into each step's PSUM accumulation
    via identity matmuls (bf16 hi+lo split, error ~2^-18).
"""

import numpy as np
import ml_dtypes

B, L, H, O, T = 64, 128, 1024, 1, 256
NC = 8
NPH = 4
# device gate-column order f|g|i|o (indices into pytorch's i,f,g,o row blocks):
# f and g first so the sigmoid splits into sig(f,g) -> DVE t1/gt starts early
# while sig(i,o) still runs on ACT.
GATE_ORDER = [1, 2, 0, 3]
# per-stream gate slices in the [128, 128] gates tile (4 gates x 32 batch)
SG_F = slice(0, 32)
SG_G = slice(32, 64)
SG_I = slice(64, 96)
SG_O = slice(96, 128)
# Filler matmuls per stream section. Besides keeping the PE p-state warm,
# they are sized so the PE is the (slightly) binding resource of each round:
# PE-paced rounds run in lockstep across all 8 cores, which kills the
# chain-bound phase oscillation (cores drifting apart and waiting on the
# slowest peer's h-broadcast costs ~1.3us/step on average).
FILLER_N = 128
FILL_A = 3
FILL_B = 1

_cache = {}


def _build_lstm_nc(T_steps=T, solo=False, detect_races=True):
    import concourse.bacc as bacc
    import concourse.bass as bass
    import concourse.mybir as mybir

    dt = mybir.dt
    AF = mybir.ActivationFunctionType
    ALU = mybir.AluOpType
    Tn = T_steps

    nc = bacc.Bacc(
        None,
        target_bir_lowering=False,
        debug=False,
        num_devices=NC,
        detect_race_conditions=detect_races,
    )

    d_latT = nc.dram_tensor("latT", [128, 64], dt.float32, kind="ExternalInput")
    d_WlinT = nc.dram_tensor("WlinT", [128, 1024], dt.float32, kind="ExternalInput")
    d_blinT = nc.dram_tensor("blinT", [128, 8], dt.float32, kind="ExternalInput")
    d_WihT = nc.dram_tensor("WihT", [128, 4096], dt.float32, kind="ExternalInput")
    d_bg = nc.dram_tensor("bg", [1, 512], dt.float32, kind="ExternalInput")
    d_ones = nc.dram_tensor("ones1", [1, 64], dt.float32, kind="ExternalInput")
    d_WhT = nc.dram_tensor("WhT", [128, 4096], dt.bfloat16, kind="ExternalInput")
    d_wout = nc.dram_tensor("wout", [128, 1], dt.bfloat16, kind="ExternalInput")
    d_I64 = nc.dram_tensor("I64", [64, 64], dt.bfloat16, kind="ExternalInput")
    d_out = nc.dram_tensor("outp", [64, Tn], dt.float32, kind="ExternalOutput")
    N_IN = 9

    s_latT = nc.alloc_sbuf_tensor("s_latT", [128, 64], dt.float32)
    s_WlinT = nc.alloc_sbuf_tensor("s_WlinT", [128, 1024], dt.float32)
    s_blinT = nc.alloc_sbuf_tensor("s_blinT", [128, 8], dt.float32)
    s_WihT = nc.alloc_sbuf_tensor("s_WihT", [128, 4096], dt.float32)
    s_bg = nc.alloc_sbuf_tensor("s_bg", [1, 512], dt.float32)
    s_ones = nc.alloc_sbuf_tensor("s_ones", [1, 64], dt.float32)
    s_WhT = nc.alloc_sbuf_tensor("s_WhT", [128, 4096], dt.bfloat16)
    s_wout = nc.alloc_sbuf_tensor("s_wout", [128, 1], dt.bfloat16)
    s_I64 = nc.alloc_sbuf_tensor("s_I64", [64, 64], dt.bfloat16)

    s_hidT = nc.alloc_sbuf_tensor("s_hidT", [128, 512], dt.float32)
    s_Xhi = nc.alloc_sbuf_tensor("s_Xhi", [64, 512], dt.bfloat16)
    s_Xlo = nc.alloc_sbuf_tensor("s_Xlo", [64, 512], dt.bfloat16)
    s_Xres = nc.alloc_sbuf_tensor("s_Xres", [64, 512], dt.float32)
    # per-stream recv buffers: 4-deep rotation, 8 slots x 32 batch cols
    recvA = [
        nc.alloc_sbuf_tensor(f"recvA{p}", [128, 256], dt.bfloat16) for p in range(NPH)
    ]
    recvB = [
        nc.alloc_sbuf_tensor(f"recvB{p}", [128, 256], dt.bfloat16) for p in range(NPH)
    ]
    gA = [nc.alloc_sbuf_tensor(f"gA{p}", [128, 128], dt.float32) for p in range(2)]
    gB = [nc.alloc_sbuf_tensor(f"gB{p}", [128, 128], dt.float32) for p in range(2)]
    thA = [nc.alloc_sbuf_tensor(f"thA{p}", [128, 32], dt.float32) for p in range(2)]
    thB = [nc.alloc_sbuf_tensor(f"thB{p}", [128, 32], dt.float32) for p in range(2)]
    # each send buffer padded to its own 512B-aligned footprint
    _hsA = [
        nc.alloc_sbuf_tensor(f"h_sendA{p}", [128, 256], dt.bfloat16) for p in range(2)
    ]
    _hsB = [
        nc.alloc_sbuf_tensor(f"h_sendB{p}", [128, 256], dt.bfloat16) for p in range(2)
    ]
    h_sendA = [t[:, 0:32] for t in _hsA]
    h_sendB = [t[:, 0:32] for t in _hsB]
    s_t1 = nc.alloc_sbuf_tensor("s_t1", [128, 32], dt.float32)
    s_t2 = nc.alloc_sbuf_tensor("s_t2", [128, 32], dt.float32)
    s_gt = nc.alloc_sbuf_tensor("s_gt", [128, 32], dt.float32)
    cA = [nc.alloc_sbuf_tensor(f"cA{p}", [128, 32], dt.float32) for p in range(2)]
    cB = [nc.alloc_sbuf_tensor(f"cB{p}", [128, 32], dt.float32) for p in range(2)]
    s_out = nc.alloc_sbuf_tensor("s_out", [64, Tn], dt.float32)

    p_hid = nc.alloc_psum_tensor("p_hid", [128, 512], dt.float32)
    p_x = nc.alloc_psum_tensor("p_x", [128, 512], dt.float32)
    p_gA = [
        nc.alloc_psum_tensor(f"p_gA{p}", [128, 512], dt.float32) for p in range(2)
    ]
    p_gB = [
        nc.alloc_psum_tensor(f"p_gB{p}", [128, 512], dt.float32) for p in range(2)
    ]
    p_out = nc.alloc_psum_tensor("p_out", [128, 512], dt.float32)
    p_fill = nc.alloc_psum_tensor("p_fill", [128, 512], dt.float32)

    s_srcA = [nc.alloc_semaphore(f"s_srcA{j}") for j in range(NC)]
    s_srcB = [nc.alloc_semaphore(f"s_srcB{j}") for j in range(NC)]
    s_peA = nc.alloc_semaphore("s_peA")
    s_peB = nc.alloc_semaphore("s_peB")
    s_sigA = nc.alloc_semaphore("s_sigA")
    s_sigB = nc.alloc_semaphore("s_sigB")
    s_thsA = nc.alloc_semaphore("s_thsA")
    s_thsB = nc.alloc_semaphore("s_thsB")
    s_cA = nc.alloc_semaphore("s_cA")
    s_cB = nc.alloc_semaphore("s_cB")
    s_hA = nc.alloc_semaphore("s_hA")
    s_hB = nc.alloc_semaphore("s_hB")
    s_locA = nc.alloc_semaphore("s_locA")
    s_locB = nc.alloc_semaphore("s_locB")
    s_prepA = nc.alloc_semaphore("s_prepA")
    s_prepB = nc.alloc_semaphore("s_prepB")
    s_ph = nc.alloc_semaphore("s_ph")
    s_v = nc.alloc_semaphore("s_v")
    s_xrdy = nc.alloc_semaphore("s_xrdy")
    s_osem = nc.alloc_semaphore("s_osem")
    s_fin = nc.alloc_semaphore("s_fin")
    dma_sem = nc.alloc_semaphore("dma_sem")

    SRC_INC = 16 if solo else 2

    # X-inject: out pg[:, 32m:32m+32] = Xhi[batch j, 128m+i] for this stream.
    def x_inject(tensor, pg, brow, istart, final_stop=False):
        for m in range(4):
            tensor.matmul(
                pg[:, 32 * m : 32 * m + 32],
                s_Xhi[brow, 128 * m : 128 * m + 128],
                s_I64[brow, istart],
                start=(m == 0),
                stop=False,
            )
            mm = tensor.matmul(
                pg[:, 32 * m : 32 * m + 32],
                s_Xlo[brow, 128 * m : 128 * m + 128],
                s_I64[brow, istart],
                start=False,
                stop=(final_stop and m == 3),
            )
        return mm

    def fillers(tensor, n):
        for fi in range(n):
            tensor.matmul(
                p_fill[:, 0:FILLER_N],
                s_WhT[:, 0:128],
                s_WhT[:, 128 : 128 + FILLER_N],
                start=(fi == 0),
                stop=(fi == n - 1),
            )

    BROW_A, IST_A = slice(0, 32), slice(0, 32)
    BROW_B, IST_B = slice(32, 64), slice(32, 64)

    with nc.Block() as block:

        @block.sync
        def _(sync: bass.BassEngine):
            for d, s in [
                (d_latT, s_latT),
                (d_WlinT, s_WlinT),
                (d_blinT, s_blinT),
                (d_WihT, s_WihT),
                (d_bg, s_bg),
                (d_ones, s_ones),
                (d_WhT, s_WhT),
                (d_wout, s_wout),
                (d_I64, s_I64),
            ]:
                sync.dma_start(s[:, :], d[:, :]).then_inc(dma_sem, 16)
            sync.wait_ge(s_fin, 1)
            sync.dma_start(d_out[:, :], s_out[:, :]).then_inc(dma_sem, 16)
            sync.wait_ge(dma_sem, 16 * (N_IN + 1))

        @block.tensor
        def _(tensor: bass.BassTensorEngine):
            tensor.wait_ge(dma_sem, 16 * N_IN)
            # phase 1a: hidden^T chunks = W_lin row-chunks @ latent^T
            for m in range(8):
                mm = tensor.matmul(
                    p_hid[:, 64 * m : 64 * m + 64],
                    s_WlinT[:, 128 * m : 128 * m + 128],
                    s_latT[:, :],
                    start=True,
                    stop=True,
                )
            mm.then_inc(s_ph, 1)  # s_ph = 1
            # phase 1b: x_gates (B-major) = hidden @ W_ih_slice^T + bias
            tensor.wait_ge(s_ph, 2)
            for k in range(8):
                tensor.matmul(
                    p_x[0:64, :],
                    s_hidT[:, 64 * k : 64 * k + 64],
                    s_WihT[:, 512 * k : 512 * k + 512],
                    start=(k == 0),
                    stop=False,
                )
            mm = tensor.matmul(
                p_x[0:64, :], s_ones[0:1, :], s_bg[0:1, :], start=False, stop=True
            )
            mm.then_inc(s_ph, 1)  # s_ph = 3
            # HAM warmup
            for fi in range(12):
                tensor.matmul(
                    p_fill[:, 0:512],
                    s_WhT[:, 0:128],
                    s_WhT[:, 128:640],
                    start=(fi == 0),
                    stop=(fi == 11),
                )
            # prologue: round-0 gates = X only
            tensor.wait_ge(s_xrdy, 1)
            x_inject(tensor, p_gA[0], BROW_A, IST_A, final_stop=True).then_inc(
                s_peA, 1
            )
            x_inject(tensor, p_gB[0], BROW_B, IST_B, final_stop=True).then_inc(
                s_peB, 1
            )

            for r in range(Tn):
                # ---- stream A ----
                if r >= 1:
                    par = r % NPH
                    pg = p_gA[r % 2]
                    for x in range(8):
                        tensor.wait_ge(s_srcA[x], SRC_INC * r)
                        for m in range(4):
                            mm = tensor.matmul(
                                pg[:, 32 * m : 32 * m + 32],
                                s_WhT[:, (4 * x + m) * 128 : (4 * x + m + 1) * 128],
                                recvA[par][:, 32 * x : 32 * x + 32],
                                start=False,
                                stop=(x == 7 and m == 3),
                            )
                    mm.then_inc(s_peA, 1)  # r+1
                if r + 1 < Tn:
                    # X for round r+1 opens the pg[(r+1)%2] accumulation group
                    x_inject(tensor, p_gA[(r + 1) % 2], BROW_A, IST_A)
                if r >= 1:
                    tensor.wait_ge(s_hA, r)
                    tensor.matmul(
                        p_out[0:32, r - 1 : r],
                        h_sendA[r % 2],
                        s_wout[:, 0:1],
                        start=True,
                        stop=True,
                    )
                fillers(tensor, FILL_A)
                # ---- stream B ----
                if r >= 1:
                    par = r % NPH
                    pg = p_gB[r % 2]
                    for x in range(8):
                        tensor.wait_ge(s_srcB[x], SRC_INC * r)
                        for m in range(4):
                            mm = tensor.matmul(
                                pg[:, 32 * m : 32 * m + 32],
                                s_WhT[:, (4 * x + m) * 128 : (4 * x + m + 1) * 128],
                                recvB[par][:, 32 * x : 32 * x + 32],
                                start=False,
                                stop=(x == 7 and m == 3),
                            )
                    mm.then_inc(s_peB, 1)  # r+1
                if r + 1 < Tn:
                    x_inject(tensor, p_gB[(r + 1) % 2], BROW_B, IST_B)
                if r >= 1:
                    tensor.wait_ge(s_hB, r)
                    tensor.matmul(
                        p_out[32:64, r - 1 : r],
                        h_sendB[r % 2],
                        s_wout[:, 0:1],
                        start=True,
                        stop=True,
                    )
                fillers(tensor, FILL_B)

            tensor.wait_ge(s_hA, Tn)
            tensor.matmul(
                p_out[0:32, Tn - 1 : Tn],
                h_sendA[Tn % 2],
                s_wout[:, 0:1],
                start=True,
                stop=True,
            ).then_inc(s_osem, 1)
            tensor.wait_ge(s_hB, Tn)
            tensor.matmul(
                p_out[32:64, Tn - 1 : Tn],
                h_sendB[Tn % 2],
                s_wout[:, 0:1],
                start=True,
                stop=True,
            ).then_inc(s_osem, 1)

        @block.scalar
        def _(scalar: bass.BassScalarEngine):
            scalar.wait_ge(s_ph, 1)
            for m in range(8):
                a = scalar.activation(
                    s_hidT[:, 64 * m : 64 * m + 64],
                    p_hid[:, 64 * m : 64 * m + 64],
                    AF.Identity,
                    bias=s_blinT[:, m : m + 1],
                    scale=1.0,
                )
            a.then_inc(s_ph, 1)  # s_ph = 2
            for r in range(Tn):
                scalar.wait_ge(s_peA, r + 1)
                # sig(f,g) first so DVE can start t1/gt while sig(i,o) runs
                scalar.activation(
                    gA[r % 2][:, 0:64], p_gA[r % 2][:, 0:64], AF.Sigmoid
                ).then_inc(s_sigA, 1)  # 2r+1
                scalar.activation(
                    gA[r % 2][:, 64:128], p_gA[r % 2][:, 64:128], AF.Sigmoid
                ).then_inc(s_sigA, 1)  # 2r+2
                scalar.wait_ge(s_cA, r + 1)
                scalar.activation(
                    thA[r % 2][:, :], cA[r % 2][:, :], AF.Tanh
                ).then_inc(s_thsA, 1)  # r+1
                scalar.wait_ge(s_peB, r + 1)
                scalar.activation(
                    gB[r % 2][:, 0:64], p_gB[r % 2][:, 0:64], AF.Sigmoid
                ).then_inc(s_sigB, 1)  # 2r+1
                scalar.activation(
                    gB[r % 2][:, 64:128], p_gB[r % 2][:, 64:128], AF.Sigmoid
                ).then_inc(s_sigB, 1)  # 2r+2
                scalar.wait_ge(s_cB, r + 1)
                scalar.activation(
                    thB[r % 2][:, :], cB[r % 2][:, :], AF.Tanh
                ).then_inc(s_thsB, 1)  # r+1
            scalar.wait_ge(s_osem, 2)
            scalar.activation(s_out[:, :], p_out[0:64, 0:Tn], AF.Copy).then_inc(
                s_fin, 1
            )

        @block.vector
        def _(vector: bass.BassVectorEngine):
            vector.wait_ge(s_ph, 3)
            vector.tensor_copy(s_Xhi[:, :], p_x[0:64, :]).then_inc(s_v, 1)
            vector.wait_ge(s_v, 1)
            vector.tensor_tensor(
                s_Xres[0:64, :], p_x[0:64, :], s_Xhi[:, :], ALU.subtract
            ).then_inc(s_v, 1)
            vector.wait_ge(s_v, 2)
            vector.tensor_copy(s_Xlo[:, :], s_Xres[0:64, :])
            vector.memset(cA[1][:, :], 0.0)
            vector.memset(cB[1][:, :], 0.0).then_inc(s_xrdy, 1)
            # intra-DVE RAW edges (gt->t2, t2->c) carry explicit self-sems:
            # back-to-back DVE ops can read an operand before the prior op's
            # write fully lands (seen as stream-A corruption without these;
            # relying on op-order spacing alone also fails on HW).
            for r in range(Tn):
                # ---- stream A cell ----
                g = gA[r % 2]
                vector.wait_ge(s_sigA, 2 * r + 1)  # sig(f,g)
                vector.tensor_tensor(
                    s_t1[:, :], g[:, SG_F], cA[(r + 1) % 2][:, :], ALU.mult
                ).then_inc(s_v, 1)  # 6r+3
                vector.tensor_scalar(
                    s_gt[:, :], g[:, SG_G], 2.0, -1.0, ALU.mult, ALU.add
                ).then_inc(s_v, 1)  # 6r+4
                vector.wait_ge(s_sigA, 2 * r + 2)  # sig(i,o)
                vector.wait_ge(s_v, 6 * r + 4)
                vector.tensor_tensor(
                    s_t2[:, :], g[:, SG_I], s_gt[:, :], ALU.mult
                ).then_inc(s_v, 1)  # 6r+5
                vector.wait_ge(s_v, 6 * r + 5)
                vector.tensor_tensor(
                    cA[r % 2][:, :], s_t1[:, :], s_t2[:, :], ALU.add
                ).then_inc(s_cA, 1)  # r+1
                vector.wait_ge(s_thsA, r + 1)
                if r >= 2 and not solo:
                    vector.wait_ge(s_locA, 16 * (r - 1))
                vector.tensor_tensor(
                    h_sendA[(r + 1) % 2], g[:, SG_O], thA[r % 2][:, :], ALU.mult
                ).then_inc(s_hA, 1)  # r+1
                # ---- stream B cell ----
                g = gB[r % 2]
                vector.wait_ge(s_sigB, 2 * r + 1)
                vector.tensor_tensor(
                    s_t1[:, :], g[:, SG_F], cB[(r + 1) % 2][:, :], ALU.mult
                ).then_inc(s_v, 1)  # 6r+6
                vector.tensor_scalar(
                    s_gt[:, :], g[:, SG_G], 2.0, -1.0, ALU.mult, ALU.add
                ).then_inc(s_v, 1)  # 6r+7
                vector.wait_ge(s_sigB, 2 * r + 2)
                vector.wait_ge(s_v, 6 * r + 7)
                vector.tensor_tensor(
                    s_t2[:, :], g[:, SG_I], s_gt[:, :], ALU.mult
                ).then_inc(s_v, 1)  # 6r+8
                vector.wait_ge(s_v, 6 * r + 8)
                vector.tensor_tensor(
                    cB[r % 2][:, :], s_t1[:, :], s_t2[:, :], ALU.add
                ).then_inc(s_cB, 1)  # r+1
                vector.wait_ge(s_thsB, r + 1)
                if r >= 2 and not solo:
                    vector.wait_ge(s_locB, 16 * (r - 1))
                vector.tensor_tensor(
                    h_sendB[(r + 1) % 2], g[:, SG_O], thB[r % 2][:, :], ALU.mult
                ).then_inc(s_hB, 1)  # r+1

        @block.gpsimd
        def _(gpsimd: bass.BassGpSimd):
            if solo:
                for r in range(Tn):
                    gpsimd.wait_ge(s_hA, r + 1)
                    for j in range(8):
                        gpsimd.dma_start(
                            recvA[(r + 1) % NPH][:, 32 * j : 32 * j + 32],
                            h_sendA[(r + 1) % 2],
                        ).then_inc(s_srcA[j], 16)
                    gpsimd.wait_ge(s_hB, r + 1)
                    for j in range(8):
                        gpsimd.dma_start(
                            recvB[(r + 1) % NPH][:, 32 * j : 32 * j + 32],
                            h_sendB[(r + 1) % 2],
                        ).then_inc(s_srcB[j], 16)
                return
            gpsimd.bir_kernel_barrier_wait([list(range(NC))])
            pid = gpsimd.partition_id()
            for case in gpsimd.Switch(pid, NC):
                rdests = [(0, j) for j in range(NC)]
                # prologue: prep round-0 frames (A then B, strict FIFO order)
                gpsimd.remote_dma_broadcast(
                    out_ap=recvA[1][:, 32 * case : 32 * case + 32],
                    in_ap=h_sendA[1],
                    remote_sem=s_srcA[case],
                    local_sem=s_locA,
                    rdests=rdests,
                ).then_inc(s_prepA, 1)
                gpsimd.remote_dma_broadcast(
                    out_ap=recvB[1][:, 32 * case : 32 * case + 32],
                    in_ap=h_sendB[1],
                    remote_sem=s_srcB[case],
                    local_sem=s_locB,
                    rdests=rdests,
                ).then_inc(s_prepB, 1)
                for r in range(Tn):
                    gpsimd.wait_ge(s_prepA, r + 1)
                    gpsimd.wait_ge(s_hA, r + 1)
                    gpsimd.trigger_dma(count=1)  # fires frame A(r)
                    gpsimd.wait_ge(s_prepB, r + 1)
                    gpsimd.wait_ge(s_hB, r + 1)
                    gpsimd.trigger_dma(count=1)  # fires frame B(r)
                    if r + 1 < Tn:
                        gpsimd.remote_dma_broadcast(
                            out_ap=recvA[(r + 2) % NPH][
                                :, 32 * case : 32 * case + 32
                            ],
                            in_ap=h_sendA[(r + 2) % 2],
                            remote_sem=s_srcA[case],
                            local_sem=s_locA,
                            rdests=rdests,
                        ).then_inc(s_prepA, 1)
                        gpsimd.remote_dma_broadcast(
                            out_ap=recvB[(r + 2) % NPH][
                                :, 32 * case : 32 * case + 32
                            ],
                            in_ap=h_sendB[(r + 2) % 2],
                            remote_sem=s_srcB[case],
                            local_sem=s_locB,
                            rdests=rdests,
                        ).then_inc(s_prepB, 1)
                    gpsimd.wait_ge(s_locA, 16 * r)
                    gpsimd.wait_ge(s_locB, 16 * r)

    nc.has_collectives = not solo
    nc.finalize()
    return nc


def _prep_core_inputs(inputs: dict, r: int, src_row=None) -> dict:
    if src_row is None:
        src_row = list(range(8))  # slot j holds logical core j's H-chunk
    f32 = np.float32
    bf16 = ml_dtypes.bfloat16
    latent = np.asarray(inputs["latent"], f32)
    W_lin = np.asarray(inputs["W_lin"], f32)
    b_lin = np.asarray(inputs["b_lin"], f32)
    W_ih = np.asarray(inputs["W_ih"], f32)
    W_hh = np.asarray(inputs["W_hh"], f32)
    b_ih = np.asarray(inputs["b_ih"], f32)
    b_hh = np.asarray(inputs["b_hh"], f32)
    W_out = np.asarray(inputs["W_out"], f32)

    HS = 128
    sl = slice(HS * r, HS * (r + 1))

    # g-gate (pytorch index 2) rows scaled by 2: tanh(x) = 2*sigmoid(2x)-1,
    # so the device applies one sigmoid to all four gates and DVE fixes g up.
    gscale = {2: 2.0}

    Wih_sl = np.concatenate(
        [
            gscale.get(g, 1.0) * W_ih[g * H + HS * r : g * H + HS * (r + 1), :]
            for g in GATE_ORDER
        ],
        axis=0,
    )
    WihT = Wih_sl.T.reshape(8, 128, 512).transpose(1, 0, 2).reshape(128, 4096).copy()

    bgv = b_ih + b_hh
    bg = np.concatenate(
        [
            gscale.get(g, 1.0) * bgv[g * H + HS * r : g * H + HS * (r + 1)]
            for g in GATE_ORDER
        ]
    ).reshape(1, 512)

    WhT = np.zeros((128, 4096), f32)
    for x in range(8):
        srcc = src_row[x]
        for m, g in enumerate(GATE_ORDER):
            blk = W_hh[
                g * H + HS * r : g * H + HS * (r + 1), HS * srcc : HS * (srcc + 1)
            ]
            WhT[:, (4 * x + m) * 128 : (4 * x + m + 1) * 128] = gscale.get(
                g, 1.0
            ) * blk.T

    return {
        "latT": np.ascontiguousarray(latent.T),
        "WlinT": np.ascontiguousarray(W_lin.T),
        "blinT": np.ascontiguousarray(b_lin.reshape(8, 128).T),
        "WihT": WihT,
        "bg": bg,
        "ones1": np.ones((1, 64), f32),
        "WhT": WhT.astype(bf16),
        "wout": np.ascontiguousarray(W_out[0, sl].reshape(128, 1)).astype(bf16),
        "I64": np.eye(64, dtype=f32).astype(bf16),
    }


def _run(inputs: dict, trace: bool = False):
    from concourse.bass_utils import run_bass_kernel_spmd

    if "nc" not in _cache:
        _cache["nc"] = _build_lstm_nc(T)
    nc = _cache["nc"]
    in_maps = [_prep_core_inputs(inputs, r) for r in range(NC)]
    res = run_bass_kernel_spmd(
        nc, in_maps, core_ids=list(range(NC)), trace=trace
    )
    outs = [np.asarray(res.results[r]["outp"], np.float64) for r in range(NC)]
    b_out = np.asarray(inputs["b_out"], np.float64)
    total = outs[0]
    for o in outs[1:]:
        total = total + o
    total = total + b_out[0]
    out = total[:, :, None].astype(np.float32)
    return out, res


def kernel(**inputs) -> np.ndarray:
    seq_len = int(inputs.get("seq_len", T))
    assert seq_len == T, f"kernel hardcoded for seq_len={T}, got {seq_len}"
    out, _ = _run(inputs, trace=False)
    return out
